# revision 5
# baseline (speedup 1.0000x reference)
"""Trainium2 Bass kernel for a dense transformer block (b=2, t=2048, d_m=1024,
h=16, d_ff=4096, causal attention, RMS norms, fp32 I/O).

Sharding over 8 NeuronCores: data-parallel over the 2 batches x 4-way tensor
parallel (attention heads + a token-sharded FFN fed by a per-512-token-block
ReduceScatter after W_O). All matmuls run in bf16 with fp32 PSUM accumulation;
the residual path stays fp32.
"""
import sys

try:
    import concourse  # noqa: F401
except ImportError:  # pragma: no cover
    sys.path.insert(0, "/opt/trn_rl_repo")

import numpy as np
import ml_dtypes

import concourse.mybir as mybir
import concourse.tile as tile
from concourse import bacc
from concourse.bass_utils import run_bass_kernel_spmd

bf16 = ml_dtypes.bfloat16
F32 = mybir.dt.float32
BF = mybir.dt.bfloat16
EXP = mybir.ActivationFunctionType.Exp
SQRT = mybir.ActivationFunctionType.Sqrt
RELU = mybir.ActivationFunctionType.Relu

NUM_CORES = 8
GROUPS = [[0, 1, 2, 3], [4, 5, 6, 7]]
T = 2048        # tokens per batch
DM = 1024       # model dim
DFF = 4096      # ffn hidden
NH_CORE = 4     # heads per core
DK = 64
EPS = 1e-6

_CACHED = {}


def _build_program():
    nc = bacc.Bacc("TRN2", target_bir_lowering=False, debug=False,
                   num_devices=NUM_CORES)

    def din(name, shape, dt=BF):
        return nc.dram_tensor(name, shape, dt, kind="ExternalInput").ap()

    xt_d = din("xt", [128, 8, T])            # X[bi].T partition-tiled (bf16)
    xc_d = din("xc", [128, 8, 512], F32)     # X chunk (fragmented tokens), fp32
    wq_d = din("wq", [128, 2, 8, 128])       # Q weights lhsT tiles
    wk_d = din("wk", [128, 2, 8, 128])
    wv_d = din("wv", [128, 8, 256])
    wo_d = din("wo", [128, 2, DM])           # W_O rows for my heads
    w1_d = din("w1", [128, 32, 8, 128])      # Wff_in lhsT tiles  [p, mg, kg, m]
    w2_d = din("w2", [128, 8, 32, 128])      # Wff_out lhsT tiles [p, mg, kg, m]
    mask_d = din("mask", [128, 4, 512])      # causal diag masks
    out_d = nc.dram_tensor("out", [128, 8, 512], F32, kind="ExternalOutput").ap()

    with tile.TileContext(nc) as tc:
        with (
            tc.tile_pool(name="res", bufs=1) as res,        # resident singletons
            tc.tile_pool(name="big", bufs=1) as big,        # xt/ht recycled slot
            tc.tile_pool(name="wst", bufs=2) as wst,        # streamed weights
            tc.tile_pool(name="tmp", bufs=3) as tmp,
            tc.tile_pool(name="stg", bufs=2) as stgp,        # transient sbuf
            tc.tile_pool(name="et", bufs=6) as etp,         # exp tiles
            tc.tile_pool(name="ps", bufs=4, space="PSUM") as ps,
            tc.tile_pool(name="pav", bufs=2, space="PSUM") as pavp,
            tc.tile_pool(name="dram", bufs=1, space="DRAM") as dram,
        ):
            # ---------- resident tiles ----------
            xt = big.tile([128, 8, T], BF, tag="big")       # becomes y1 in place
            qt = res.tile([128, 2, T], BF)
            ks = res.tile([128, 4, T], BF)                  # zero-padded K by parity
            vv = res.tile([128, 16, NH_CORE, DK + 1], BF)   # V' with ones col
            attnT = res.tile([128, 2, T], BF)               # packed attn out
            rb1 = res.tile([128, T], BF)
            rb2 = res.tile([128, 512], BF)
            x2 = res.tile([128, 8, 512], F32)
            y2 = res.tile([128, 8, 512], BF)
            ot = res.tile([128, 8, 512], BF)                # RS readback (O chunk)
            masks = res.tile([128, 4, 512], BF)
            wqs = res.tile([128, 2, 8, 128], BF)
            wks = res.tile([128, 2, 8, 128], BF)
            wvs = res.tile([128, 8, 256], BF)
            wos = res.tile([128, 2, DM], BF)
            ones = res.tile([128, 128], BF)
            eps_ap = res.tile([128, 1], F32)

            # DRAM bounce buffers for the 4 pipelined ReduceScatters
            rs_in = [dram.tile([512, DM], BF, name=f"rs_in{i}") for i in range(4)]
            rs_out = [dram.tile([128, DM], BF, name=f"rs_out{i}") for i in range(4)]

            # ---------- input DMAs + inits ----------
            nc.sync.dma_start(xt[:], xt_d[:])
            nc.sync.dma_start(wqs[:], wq_d[:])
            nc.sync.dma_start(wks[:], wk_d[:])
            nc.sync.dma_start(wvs[:], wv_d[:])
            nc.sync.dma_start(wos[:], wo_d[:])
            nc.sync.dma_start(masks[:], mask_d[:])
            nc.vector.memset(ones[:], 1.0)
            nc.vector.memset(eps_ap[:], EPS)
            nc.vector.memset(ks[:], 0.0)
            nc.vector.memset(vv[:, :, :, DK:DK + 1], 1.0)

            # ---------- stats1: rb1 = 1/sqrt(mean(x^2)+eps), bcast on parts ----
            for ts in range(4):
                st = ps.tile([128, 512], F32, tag="mm")
                for kg in range(8):
                    sq = tmp.tile([128, 512], BF, tag="sq")
                    xs = xt[:, kg, ts * 512:(ts + 1) * 512]
                    nc.vector.tensor_mul(sq[:], xs, xs)
                    nc.tensor.matmul(st[:], ones[:], sq[:],
                                     start=(kg == 0), stop=(kg == 7))
                srt = tmp.tile([128, 512], F32, tag="srt")
                nc.scalar.activation(srt[:], st[:], SQRT,
                                     scale=1.0 / DM, bias=eps_ap[:])
                with nc.allow_low_precision(reason="bf16 rms scale"):
                    nc.vector.reciprocal(rb1[:, ts * 512:(ts + 1) * 512], srt[:])

            # ---------- y1 = xt * rb1 (in place) ----------
            for kg in range(8):
                nc.vector.tensor_mul(xt[:, kg, :], xt[:, kg, :], rb1[:])
            y1 = xt

            # ---------- QKV projections ----------
            # Q: psum [128(2 heads), 512] per (fg, ts); K same into padded ks
            for fg in range(2):
                for ts in range(4):
                    qp = ps.tile([128, 512], F32, tag="mm")
                    for kg in range(8):
                        nc.tensor.matmul(qp[:], wqs[:, fg, kg, :],
                                         y1[:, kg, ts * 512:(ts + 1) * 512],
                                         start=(kg == 0), stop=(kg == 7))
                    nc.vector.tensor_copy(qt[:, fg, ts * 512:(ts + 1) * 512], qp[:])
                    kp = ps.tile([128, 512], F32, tag="mm")
                    for kg in range(8):
                        nc.tensor.matmul(kp[:], wks[:, fg, kg, :],
                                         y1[:, kg, ts * 512:(ts + 1) * 512],
                                         start=(kg == 0), stop=(kg == 7))
                    sl = slice(ts * 512, (ts + 1) * 512)
                    nc.vector.tensor_copy(ks[0:64, 2 * fg, sl], kp[0:64, :])
                    nc.vector.tensor_copy(ks[64:128, 2 * fg + 1, sl], kp[64:128, :])
            # V: token-partition layout via lhsT = y1 tile
            for tt in range(16):
                vp = ps.tile([128, 512], F32, tag="mm", name="vp")[:, 0:256]
                for kg in range(8):
                    nc.tensor.matmul(vp[:], y1[:, kg, tt * 128:(tt + 1) * 128],
                                     wvs[:, kg, :],
                                     start=(kg == 0), stop=(kg == 7))
                nc.vector.tensor_copy(
                    vv[:, tt, :, 0:DK],
                    vp.rearrange("p (h d) -> p h d", h=NH_CORE))

            # ---------- attention + per-qg W_O partial + ReduceScatter ----------
            for qg in range(4):
                qsl = slice(qg * 512, (qg + 1) * 512)
                for h in range(NH_CORE):
                    j, par = divmod(h, 2)
                    av = pavp.tile([DK + 1, 512], F32, tag="av")
                    nkt = 4 * qg + 4
                    for kt in range(nkt):
                        sc = ps.tile([128, 512], F32, tag="mm")
                        nc.tensor.matmul(sc[:], ks[:, h, kt * 128:(kt + 1) * 128],
                                         qt[:, j, qsl], start=True, stop=True)
                        et = etp.tile([128, 512], BF, tag="et")
                        nc.scalar.activation(et[:], sc[:], EXP, scale=0.125)
                        off = kt - 4 * qg
                        if off >= 0:
                            nc.vector.tensor_mul(et[:], et[:], masks[:, off, :])
                        nc.tensor.matmul(av[:], vv[:, kt, h, :], et[:],
                                         start=(kt == 0), stop=(kt == nkt - 1))
                    rsb = tmp.tile([1, 512], BF, tag="rsb")
                    with nc.allow_low_precision(reason="bf16 softmax denom"):
                        nc.vector.reciprocal(rsb[0:1, :], av[DK:DK + 1, :])
                    prb = ps.tile([128, 512], F32, tag="mm", name="prb")[0:64, :]
                    nc.tensor.matmul(prb[:], ones[0:1, 0:64], rsb[0:1, :],
                                     start=True, stop=True)
                    dsb = tmp.tile([64, 512], F32, tag="dsb")
                    nc.vector.tensor_copy(dsb[:], prb[:])
                    pb = 64 * par
                    nc.vector.tensor_mul(attnT[pb:pb + 64, j, qsl],
                                         av[0:DK, :], dsb[:])
                # W_O partial for this qg, transposed to token rows
                for tsub in range(4):
                    tt = 4 * qg + tsub
                    stg = stgp.tile([128, DM], BF, tag="ostg")
                    for nb in range(2):
                        op = ps.tile([128, 512], F32, tag="mm")
                        for j in range(2):
                            nc.tensor.matmul(
                                op[:], attnT[:, j, tt * 128:(tt + 1) * 128],
                                wos[:, j, nb * 512:(nb + 1) * 512],
                                start=(j == 0), stop=(j == 1))
                        nc.vector.tensor_copy(stg[:, nb * 512:(nb + 1) * 512], op[:])
                    nc.sync.dma_start(rs_in[qg][tsub * 128:(tsub + 1) * 128, :],
                                      stg[:])
                nc.gpsimd.collective_compute(
                    "ReduceScatter", mybir.AluOpType.add,
                    replica_groups=GROUPS,
                    ins=[rs_in[qg].opt()], outs=[rs_out[qg].opt()])
                # transposed readback: [128 tok, 1024 dout] -> ot[:, kg, qg*128..]
                for kg in range(8):
                    nc.sync.dma_start(
                        ot[:, kg, qg * 128:(qg + 1) * 128],
                        rs_out[qg][:, kg * 128:(kg + 1) * 128],
                        transpose=True)

            # ---------- residual + stats2 + y2 ----------
            st2 = ps.tile([128, 512], F32, tag="mm")
            for kg in range(8):
                xck = tmp.tile([128, 512], F32, tag="xck")
                nc.sync.dma_start(xck[:], xc_d[:, kg, :])
                nc.vector.tensor_add(x2[:, kg, :], xck[:], ot[:, kg, :])
                sq = tmp.tile([128, 512], BF, tag="sq")
                nc.vector.tensor_mul(sq[:], x2[:, kg, :], x2[:, kg, :])
                nc.tensor.matmul(st2[:], ones[:], sq[:],
                                 start=(kg == 0), stop=(kg == 7))
            srt2 = tmp.tile([128, 512], F32, tag="srt")
            nc.scalar.activation(srt2[:], st2[:], SQRT,
                                 scale=1.0 / DM, bias=eps_ap[:])
            with nc.allow_low_precision(reason="bf16 rms scale"):
                nc.vector.reciprocal(rb2[:], srt2[:])
            for kg in range(8):
                nc.vector.tensor_copy(y2[:, kg, :], x2[:, kg, :])
                nc.vector.tensor_mul(y2[:, kg, :], y2[:, kg, :], rb2[:])

            # ---------- FFN1: ht = relu(W1^T y2) ----------
            ht = big.tile([128, 32, 512], BF, tag="big")    # recycles xt slot
            for mg in range(32):
                w1t = wst.tile([128, 8, 128], BF, tag="w1")
                nc.sync.dma_start(w1t[:], w1_d[:, mg, :, :])
                hp = ps.tile([128, 512], F32, tag="mm")
                for kg in range(8):
                    nc.tensor.matmul(hp[:], w1t[:, kg, :], y2[:, kg, :],
                                     start=(kg == 0), stop=(kg == 7))
                nc.scalar.activation(ht[:, mg, :], hp[:], RELU)

            # ---------- FFN2 + final residual ----------
            for mg in range(8):
                w2t = wst.tile([128, 32, 128], BF, tag="w2")
                nc.sync.dma_start(w2t[:], w2_d[:, mg, :, :])
                fp = ps.tile([128, 512], F32, tag="mm")
                for kg in range(32):
                    nc.tensor.matmul(fp[:], w2t[:, kg, :], ht[:, kg, :],
                                     start=(kg == 0), stop=(kg == 31))
                nc.vector.tensor_add(x2[:, mg, :], x2[:, mg, :], fp[:])
            nc.sync.dma_start(out_d[:], x2[:])

    nc.compile()
    return nc


def _part_tile(a2d):
    """[N*128, M] -> [128, N, M] with [p, n, m] = a[n*128+p, m]."""
    n = a2d.shape[0] // 128
    return np.ascontiguousarray(
        a2d.reshape(n, 128, a2d.shape[1]).transpose(1, 0, 2))


def _chunk_tokens(g):
    c = np.arange(512)
    return 512 * (c // 128) + 128 * g + (c % 128)


def kernel(X, W_QKV, W_O, Wff_in, Wff_out, gamma1, gamma2):
    X = np.asarray(X, dtype=np.float32)
    W_QKV = np.asarray(W_QKV, dtype=np.float32)
    W_O = np.asarray(W_O, dtype=np.float32)
    Wff_in = np.asarray(Wff_in, dtype=np.float32)
    Wff_out = np.asarray(Wff_out, dtype=np.float32)
    gamma1 = np.asarray(gamma1, dtype=np.float32)
    gamma2 = np.asarray(gamma2, dtype=np.float32)
    b = X.shape[0]

    if "nc" not in _CACHED:
        _CACHED["nc"] = _build_program()
    nc = _CACHED["nc"]

    Wq = (gamma1[:, None] * W_QKV[:, 0:DM]).astype(bf16)
    Wk = (gamma1[:, None] * W_QKV[:, DM:2 * DM]).astype(bf16)
    Wv = (gamma1[:, None] * W_QKV[:, 2 * DM:3 * DM]).astype(bf16)
    W1 = (gamma2[:, None] * Wff_in).astype(bf16)
    w1_all = _part_tile(W1.reshape(DM, DFF)).reshape(128, 8, 32, 128)
    w1_all = np.ascontiguousarray(w1_all.transpose(0, 2, 1, 3))  # [p, mg, kg, m]
    w2f = Wff_out.astype(bf16)
    w2_all = _part_tile(w2f).reshape(128, 32, 8, 128)
    w2_all = np.ascontiguousarray(w2_all.transpose(0, 2, 1, 3))  # [p, mg, kg, m]
    wo_bf = W_O.astype(bf16)

    i0, i1 = np.indices((128, 512))
    mask_np = np.stack([(i1 >= i0 + 128 * off) for off in range(4)], axis=1)
    mask_np = mask_np.astype(bf16)  # [128, 4, 512]

    in_maps = []
    for r in range(NUM_CORES):
        bi, g = divmod(r, 4)
        Xb = X[bi]
        toks = _chunk_tokens(g)
        xt_np = _part_tile(np.ascontiguousarray(Xb.T).astype(bf16))
        xc_np = _part_tile(np.ascontiguousarray(Xb[toks].T.astype(np.float32)))
        hsl = slice(256 * g, 256 * (g + 1))
        wq_np = _part_tile(Wq[:, hsl]).reshape(128, 8, 2, 128)
        wq_np = np.ascontiguousarray(wq_np.transpose(0, 2, 1, 3))
        wk_np = _part_tile(Wk[:, hsl]).reshape(128, 8, 2, 128)
        wk_np = np.ascontiguousarray(wk_np.transpose(0, 2, 1, 3))
        wv_np = _part_tile(Wv[:, hsl])
        wo_np = np.ascontiguousarray(
            wo_bf[hsl, :].reshape(2, 128, DM).transpose(1, 0, 2))
        in_maps.append(dict(
            xt=xt_np, xc=xc_np, wq=wq_np, wk=wk_np, wv=wv_np, wo=wo_np,
            w1=w1_all, w2=w2_all, mask=mask_np))

    res = run_bass_kernel_spmd(nc, in_maps, list(range(NUM_CORES)))

    out = np.zeros_like(X)
    for r in range(NUM_CORES):
        bi, g = divmod(r, 4)
        toks = _chunk_tokens(g)
        ft = res.results[r]["out"].transpose(1, 0, 2).reshape(DM, 512)
        out[bi, toks, :] = ft.T
    return out


# revision 6
# speedup vs baseline: 1.1071x; 1.1071x over previous
"""Trainium2 Bass kernel for a dense transformer block (b=2, t=2048, d_m=1024,
h=16, d_ff=4096, causal attention, RMS norms, fp32 I/O).

Sharding over 8 NeuronCores: data-parallel over the 2 batches x 4-way tensor
parallel (attention heads + a token-sharded FFN fed by a per-512-token-block
ReduceScatter after W_O). All matmuls run in bf16 with fp32 PSUM accumulation;
the residual path stays fp32.
"""
import sys

try:
    import concourse  # noqa: F401
except ImportError:  # pragma: no cover
    sys.path.insert(0, "/opt/trn_rl_repo")

import numpy as np
import ml_dtypes

import concourse.mybir as mybir
import concourse.tile as tile
from concourse import bacc
from concourse.bass_utils import run_bass_kernel_spmd

bf16 = ml_dtypes.bfloat16
F32 = mybir.dt.float32
BF = mybir.dt.bfloat16
EXP = mybir.ActivationFunctionType.Exp
SQRT = mybir.ActivationFunctionType.Sqrt
RELU = mybir.ActivationFunctionType.Relu

NUM_CORES = 8
GROUPS = [[0, 1, 2, 3], [4, 5, 6, 7]]
T = 2048        # tokens per batch
DM = 1024       # model dim
DFF = 4096      # ffn hidden
NH_CORE = 4     # heads per core
DK = 64
EPS = 1e-6

_CACHED = {}


def _build_program():
    nc = bacc.Bacc("TRN2", target_bir_lowering=False, debug=False,
                   num_devices=NUM_CORES)

    def din(name, shape, dt=BF):
        return nc.dram_tensor(name, shape, dt, kind="ExternalInput").ap()

    xt_d = din("xt", [128, 8, T])            # X[bi].T partition-tiled (bf16)
    xc_d = din("xc", [128, 8, 512], F32)     # X chunk (fragmented tokens), fp32
    wq_d = din("wq", [128, 2, 8, 128])       # Q weights lhsT tiles
    wk_d = din("wk", [128, 2, 8, 128])
    wv_d = din("wv", [128, 8, 256])
    wo_d = din("wo", [128, 2, DM])           # W_O rows for my heads
    w1_d = din("w1", [128, 32, 8, 128])      # Wff_in lhsT tiles  [p, mg, kg, m]
    w2_d = din("w2", [128, 8, 32, 128])      # Wff_out lhsT tiles [p, mg, kg, m]
    mask_d = din("mask", [128, 4, 512])      # causal diag masks
    out_d = nc.dram_tensor("out", [128, 8, 512], F32, kind="ExternalOutput").ap()

    with tile.TileContext(nc) as tc:
        with (
            tc.tile_pool(name="res", bufs=1) as res,        # resident singletons
            tc.tile_pool(name="big", bufs=1) as big,        # xt/ht recycled slot
            tc.tile_pool(name="wst", bufs=2) as wst,        # streamed weights
            tc.tile_pool(name="tmp", bufs=3) as tmp,
            tc.tile_pool(name="stg", bufs=2) as stgp,        # transient sbuf
            tc.tile_pool(name="et", bufs=18) as etp,         # exp tiles
            tc.tile_pool(name="ps", bufs=4, space="PSUM") as ps,
            tc.tile_pool(name="pav", bufs=3, space="PSUM") as pavp,
            tc.tile_pool(name="dram", bufs=1, space="DRAM") as dram,
        ):
            # ---------- resident tiles ----------
            xt = big.tile([128, 8, T], BF, tag="big")       # becomes y1 in place
            qt = res.tile([128, 2, T], BF)
            ks = res.tile([128, 4, T], BF)                  # zero-padded K by parity
            vv = res.tile([128, 16, NH_CORE, DK + 1], BF)   # V' with ones col
            attnT = res.tile([128, 2, T], BF)               # packed attn out
            rb1 = res.tile([128, T], BF)
            rb2 = res.tile([128, 512], BF)
            x2 = res.tile([128, 8, 512], F32)
            y2 = res.tile([128, 8, 512], BF)
            ot = res.tile([128, 8, 512], BF)                # RS readback (O chunk)
            masks = res.tile([128, 4, 512], BF)
            wqs = res.tile([128, 2, 8, 128], BF)
            wks = res.tile([128, 2, 8, 128], BF)
            wvs = res.tile([128, 8, 256], BF)
            wos = res.tile([128, 2, DM], BF)
            ones = res.tile([128, 128], BF)
            eps_ap = res.tile([128, 1], F32)

            # DRAM bounce buffers for the 4 pipelined ReduceScatters
            rs_in = [dram.tile([512, DM], BF, name=f"rs_in{i}") for i in range(4)]
            rs_out = [dram.tile([128, DM], BF, name=f"rs_out{i}") for i in range(4)]

            # ---------- input DMAs + inits ----------
            for kg in range(8):
                nc.sync.dma_start(xt[:, kg, :], xt_d[:, kg, :])
            nc.sync.dma_start(wqs[:], wq_d[:])
            nc.sync.dma_start(wks[:], wk_d[:])
            nc.sync.dma_start(wvs[:], wv_d[:])
            nc.sync.dma_start(wos[:], wo_d[:])
            nc.sync.dma_start(masks[:], mask_d[:])
            nc.vector.memset(ones[:], 1.0)
            nc.vector.memset(eps_ap[:], EPS)
            nc.vector.memset(ks[:], 0.0)
            nc.vector.memset(vv[:, :, :, DK:DK + 1], 1.0)

            # ---------- stats1: rb1 = 1/sqrt(mean(x^2)+eps), bcast on parts ----
            for ts in range(4):
                st = ps.tile([128, 512], F32, tag="mm")
                for kg in range(8):
                    sq = tmp.tile([128, 512], BF, tag="sq")
                    xs = xt[:, kg, ts * 512:(ts + 1) * 512]
                    nc.vector.tensor_mul(sq[:], xs, xs)
                    nc.tensor.matmul(st[:], ones[:], sq[:],
                                     start=(kg == 0), stop=(kg == 7))
                srt = tmp.tile([128, 512], F32, tag="srt")
                nc.scalar.activation(srt[:], st[:], SQRT,
                                     scale=1.0 / DM, bias=eps_ap[:])
                rcf = tmp.tile([128, 512], F32, tag="rcf")
                nc.vector.reciprocal_approx_fast(rcf[:], srt[:])
                nc.vector.tensor_copy(rb1[:, ts * 512:(ts + 1) * 512], rcf[:])

            # ---------- y1 = xt * rb1 (in place) ----------
            for kg in range(8):
                nc.vector.tensor_mul(xt[:, kg, :], xt[:, kg, :], rb1[:])
            y1 = xt

            # ---------- QKV projections ----------
            # Q: psum [128(2 heads), 512] per (fg, ts); K same into padded ks
            for fg in range(2):
                for ts in range(4):
                    qp = ps.tile([128, 512], F32, tag="mm")
                    for kg in range(8):
                        nc.tensor.matmul(qp[:], wqs[:, fg, kg, :],
                                         y1[:, kg, ts * 512:(ts + 1) * 512],
                                         start=(kg == 0), stop=(kg == 7))
                    nc.vector.tensor_copy(qt[:, fg, ts * 512:(ts + 1) * 512], qp[:])
                    kp = ps.tile([128, 512], F32, tag="mm")
                    for kg in range(8):
                        nc.tensor.matmul(kp[:], wks[:, fg, kg, :],
                                         y1[:, kg, ts * 512:(ts + 1) * 512],
                                         start=(kg == 0), stop=(kg == 7))
                    sl = slice(ts * 512, (ts + 1) * 512)
                    nc.vector.tensor_copy(ks[0:64, 2 * fg, sl], kp[0:64, :])
                    nc.vector.tensor_copy(ks[64:128, 2 * fg + 1, sl], kp[64:128, :])
            # V: token-partition layout via lhsT = y1 tile
            for tt in range(16):
                vp = ps.tile([128, 512], F32, tag="mm", name="vp")[:, 0:256]
                for kg in range(8):
                    nc.tensor.matmul(vp[:], y1[:, kg, tt * 128:(tt + 1) * 128],
                                     wvs[:, kg, :],
                                     start=(kg == 0), stop=(kg == 7))
                nc.vector.tensor_copy(
                    vv[:, tt, :, 0:DK],
                    vp.rearrange("p (h d) -> p h d", h=NH_CORE))

            # ---------- attention + per-qg W_O partial + ReduceScatter ----------
            for qg in range(4):
                qsl = slice(qg * 512, (qg + 1) * 512)
                for h in range(NH_CORE):
                    j, par = divmod(h, 2)
                    av = pavp.tile([DK + 1, 512], F32, tag="av")
                    nkt = 4 * qg + 4
                    ets = []
                    for kt in range(nkt):
                        sc = ps.tile([128, 512], F32, tag="mm")
                        nc.tensor.matmul(sc[:], ks[:, h, kt * 128:(kt + 1) * 128],
                                         qt[:, j, qsl], start=True, stop=True)
                        et = etp.tile([128, 512], BF, tag="et", name=f"et{kt}")
                        nc.scalar.activation(et[:], sc[:], EXP, scale=0.125)
                        off = kt - 4 * qg
                        if off >= 0:
                            nc.vector.tensor_mul(et[:], et[:], masks[:, off, :])
                        ets.append(et)
                    for kt in range(nkt):
                        nc.tensor.matmul(av[:], vv[:, kt, h, :], ets[kt][:],
                                         start=(kt == 0), stop=(kt == nkt - 1))
                    rsf = tmp.tile([1, 512], F32, tag="rsf")
                    nc.vector.reciprocal_approx_fast(rsf[0:1, :], av[DK:DK + 1, :])
                    rsb = tmp.tile([1, 512], BF, tag="rsb")
                    nc.vector.tensor_copy(rsb[0:1, :], rsf[0:1, :])
                    prb = ps.tile([128, 512], F32, tag="mm", name="prb")[0:64, :]
                    nc.tensor.matmul(prb[:], ones[0:1, 0:64], rsb[0:1, :],
                                     start=True, stop=True)
                    dsb = tmp.tile([64, 512], F32, tag="dsb")
                    nc.vector.tensor_copy(dsb[:], prb[:])
                    pb = 64 * par
                    nc.vector.tensor_mul(attnT[pb:pb + 64, j, qsl],
                                         av[0:DK, :], dsb[:])
                # W_O partial for this qg, transposed to token rows
                for tsub in range(4):
                    tt = 4 * qg + tsub
                    stg = stgp.tile([128, DM], BF, tag="ostg")
                    for nb in range(2):
                        op = ps.tile([128, 512], F32, tag="mm")
                        for j in range(2):
                            nc.tensor.matmul(
                                op[:], attnT[:, j, tt * 128:(tt + 1) * 128],
                                wos[:, j, nb * 512:(nb + 1) * 512],
                                start=(j == 0), stop=(j == 1))
                        nc.vector.tensor_copy(stg[:, nb * 512:(nb + 1) * 512], op[:])
                    nc.sync.dma_start(rs_in[qg][tsub * 128:(tsub + 1) * 128, :],
                                      stg[:])
                nc.gpsimd.collective_compute(
                    "ReduceScatter", mybir.AluOpType.add,
                    replica_groups=GROUPS,
                    ins=[rs_in[qg].opt()], outs=[rs_out[qg].opt()])
                # transposed readback: [128 tok, 1024 dout] -> ot[:, kg, qg*128..]
                for kg in range(8):
                    nc.sync.dma_start(
                        ot[:, kg, qg * 128:(qg + 1) * 128],
                        rs_out[qg][:, kg * 128:(kg + 1) * 128],
                        transpose=True)

            # ---------- residual + stats2 + y2 ----------
            st2 = ps.tile([128, 512], F32, tag="mm")
            for kg in range(8):
                xck = tmp.tile([128, 512], F32, tag="xck")
                nc.sync.dma_start(xck[:], xc_d[:, kg, :])
                nc.vector.tensor_add(x2[:, kg, :], xck[:], ot[:, kg, :])
                sq = tmp.tile([128, 512], BF, tag="sq")
                nc.vector.tensor_mul(sq[:], x2[:, kg, :], x2[:, kg, :])
                nc.tensor.matmul(st2[:], ones[:], sq[:],
                                 start=(kg == 0), stop=(kg == 7))
            srt2 = tmp.tile([128, 512], F32, tag="srt")
            nc.scalar.activation(srt2[:], st2[:], SQRT,
                                 scale=1.0 / DM, bias=eps_ap[:])
            rcf2 = tmp.tile([128, 512], F32, tag="rcf")
            nc.vector.reciprocal_approx_fast(rcf2[:], srt2[:])
            nc.vector.tensor_copy(rb2[:], rcf2[:])
            for kg in range(8):
                nc.vector.tensor_copy(y2[:, kg, :], x2[:, kg, :])
                nc.vector.tensor_mul(y2[:, kg, :], y2[:, kg, :], rb2[:])

            # ---------- FFN1: ht = relu(W1^T y2) ----------
            ht = big.tile([128, 32, 512], BF, tag="big")    # recycles xt slot
            for mg in range(32):
                w1t = wst.tile([128, 8, 128], BF, tag="w1")
                nc.sync.dma_start(w1t[:], w1_d[:, mg, :, :])
                hp = ps.tile([128, 512], F32, tag="mm")
                for kg in range(8):
                    nc.tensor.matmul(hp[:], w1t[:, kg, :], y2[:, kg, :],
                                     start=(kg == 0), stop=(kg == 7))
                nc.scalar.activation(ht[:, mg, :], hp[:], RELU)

            # ---------- FFN2 + final residual ----------
            for mg in range(8):
                w2t = wst.tile([128, 32, 128], BF, tag="w2")
                nc.sync.dma_start(w2t[:], w2_d[:, mg, :, :])
                fp = ps.tile([128, 512], F32, tag="mm")
                for kg in range(32):
                    nc.tensor.matmul(fp[:], w2t[:, kg, :], ht[:, kg, :],
                                     start=(kg == 0), stop=(kg == 31))
                nc.vector.tensor_add(x2[:, mg, :], x2[:, mg, :], fp[:])
            nc.sync.dma_start(out_d[:], x2[:])

    nc.compile()
    return nc


def _part_tile(a2d):
    """[N*128, M] -> [128, N, M] with [p, n, m] = a[n*128+p, m]."""
    n = a2d.shape[0] // 128
    return np.ascontiguousarray(
        a2d.reshape(n, 128, a2d.shape[1]).transpose(1, 0, 2))


def _chunk_tokens(g):
    c = np.arange(512)
    return 512 * (c // 128) + 128 * g + (c % 128)


def kernel(X, W_QKV, W_O, Wff_in, Wff_out, gamma1, gamma2):
    X = np.asarray(X, dtype=np.float32)
    W_QKV = np.asarray(W_QKV, dtype=np.float32)
    W_O = np.asarray(W_O, dtype=np.float32)
    Wff_in = np.asarray(Wff_in, dtype=np.float32)
    Wff_out = np.asarray(Wff_out, dtype=np.float32)
    gamma1 = np.asarray(gamma1, dtype=np.float32)
    gamma2 = np.asarray(gamma2, dtype=np.float32)
    b = X.shape[0]

    if "nc" not in _CACHED:
        _CACHED["nc"] = _build_program()
    nc = _CACHED["nc"]

    Wq = (gamma1[:, None] * W_QKV[:, 0:DM]).astype(bf16)
    Wk = (gamma1[:, None] * W_QKV[:, DM:2 * DM]).astype(bf16)
    Wv = (gamma1[:, None] * W_QKV[:, 2 * DM:3 * DM]).astype(bf16)
    W1 = (gamma2[:, None] * Wff_in).astype(bf16)
    w1_all = _part_tile(W1.reshape(DM, DFF)).reshape(128, 8, 32, 128)
    w1_all = np.ascontiguousarray(w1_all.transpose(0, 2, 1, 3))  # [p, mg, kg, m]
    w2f = Wff_out.astype(bf16)
    w2_all = _part_tile(w2f).reshape(128, 32, 8, 128)
    w2_all = np.ascontiguousarray(w2_all.transpose(0, 2, 1, 3))  # [p, mg, kg, m]
    wo_bf = W_O.astype(bf16)

    i0, i1 = np.indices((128, 512))
    mask_np = np.stack([(i1 >= i0 + 128 * off) for off in range(4)], axis=1)
    mask_np = mask_np.astype(bf16)  # [128, 4, 512]

    in_maps = []
    for r in range(NUM_CORES):
        bi, g = divmod(r, 4)
        Xb = X[bi]
        toks = _chunk_tokens(g)
        xt_np = _part_tile(np.ascontiguousarray(Xb.T).astype(bf16))
        xc_np = _part_tile(np.ascontiguousarray(Xb[toks].T.astype(np.float32)))
        hsl = slice(256 * g, 256 * (g + 1))
        wq_np = _part_tile(Wq[:, hsl]).reshape(128, 8, 2, 128)
        wq_np = np.ascontiguousarray(wq_np.transpose(0, 2, 1, 3))
        wk_np = _part_tile(Wk[:, hsl]).reshape(128, 8, 2, 128)
        wk_np = np.ascontiguousarray(wk_np.transpose(0, 2, 1, 3))
        wv_np = _part_tile(Wv[:, hsl])
        wo_np = np.ascontiguousarray(
            wo_bf[hsl, :].reshape(2, 128, DM).transpose(1, 0, 2))
        in_maps.append(dict(
            xt=xt_np, xc=xc_np, wq=wq_np, wk=wk_np, wv=wv_np, wo=wo_np,
            w1=w1_all, w2=w2_all, mask=mask_np))

    res = run_bass_kernel_spmd(nc, in_maps, list(range(NUM_CORES)))

    out = np.zeros_like(X)
    for r in range(NUM_CORES):
        bi, g = divmod(r, 4)
        toks = _chunk_tokens(g)
        ft = res.results[r]["out"].transpose(1, 0, 2).reshape(DM, 512)
        out[bi, toks, :] = ft.T
    return out
